# revision 29
# baseline (speedup 1.0000x reference)
import sys
import functools

sys.path.insert(0, "/opt/trn_rl_repo")
import numpy as np
import ml_dtypes

# Problem constants (nn_Causal_GraphConvolution): hardcoded per harness contract.
K = 2
N = 8192
IN_F = 128
OUT_F = 64
NCORES = 8
ROWS = N // NCORES   # 1024 attention rows per core
NCH = N // 128       # 64 column chunks of 128
RCH = ROWS // 128    # 8 row chunks of 128
CPG = 4              # adjacency chunks per DMA group
NGRP = NCH // CPG    # 16 adjacency tiles

ADJ_FP8 = True       # adjacency stored fp8 in DRAM, cast to bf16 on DMA
HP_FP8 = False       # h' gathered in fp8 (halves the collective)
P2_SPLIT = True      # k=0 phase 2 runs while the k=1 all-gather is in flight

# t-builds sent to ScalarE (relu path, plus a PE adj@Whp correction); the
# rest stay on VectorE as tensor_scalar. Balanced so ACT(n*1075) matches
# DVE(masks 70us + (128-n)*327 + misc) inside the phase-1 window.
N_ACT = 88


def _act_schedule():
    """on_act[k][ch]: every chunk i with floor crossing gets the ACT path,
    spread evenly so neither engine starves in any stretch."""
    on_act = [[False] * NCH for _ in range(K)]
    for i in range(K * NCH):
        if (i + 1) * N_ACT // (K * NCH) > i * N_ACT // (K * NCH):
            k, ch = i % K, i // K
            if ch != NCH - 1:
                on_act[k][ch] = True
    return on_act


@functools.lru_cache(maxsize=3)
def _build():
    import concourse.bacc as bacc
    import concourse.tile as tile
    from concourse.tile import add_dep_helper
    from concourse import mybir

    bf16 = mybir.dt.bfloat16
    f32 = mybir.dt.float32
    f8e4 = mybir.dt.float8e4
    AO = mybir.AluOpType
    AF = mybir.ActivationFunctionType
    adj_dt = f8e4 if ADJ_FP8 else bf16
    hp_dt = f8e4 if HP_FP8 else bf16

    nc = bacc.Bacc(num_devices=NCORES)

    # Per-core inputs (the SPMD in_maps supply different data per core).
    # adjT[g, p, j, r] = adj[core_rows[r], (g*CPG+j)*128 + p]
    adjT = nc.declare_dram_parameter("adjT", [NGRP, 128, CPG, ROWS], adj_dt, False)
    xT = nc.declare_dram_parameter("xT", [K, IN_F, N], bf16, False)
    xrT = nc.declare_dram_parameter("xrT", [K, IN_F, ROWS], bf16, False)
    waug = nc.declare_dram_parameter("waug", [IN_F, 66], bf16, False)
    out = nc.declare_dram_parameter("out", [ROWS, K, OUT_F], f32, True)

    urow = nc.dram_tensor("urow", [K, 1, ROWS], bf16)
    hp_local = [
        nc.dram_tensor(f"hp_local{k}", [ROWS, OUT_F], hp_dt) for k in range(K)
    ]
    hp_full = [
        nc.dram_tensor(f"hp_full{k}", [N, OUT_F], hp_dt, addr_space="Shared")
        for k in range(K)
    ]

    with tile.TileContext(nc) as tc:
        with (
            tc.tile_pool(name="persist", bufs=1) as persist,
            tc.tile_pool(name="adjp", bufs=NGRP) as adjp,
            tc.tile_pool(name="xp", bufs=1) as xp,
            tc.tile_pool(name="tp", bufs=2) as tp,
            tc.tile_pool(name="pp", bufs=2) as pp,
            tc.tile_pool(name="hpio", bufs=3) as hpio,
            tc.tile_pool(name="sm", bufs=4) as sm,
        ):
            # phase-0 inputs first so they are not queued behind the 8MB
            # adjacency stream
            waug_sb = persist.tile([IN_F, 66], bf16, tag="waug")
            nc.gpsimd.dma_start(out=waug_sb, in_=waug[:])
            neg1 = persist.tile([128, 1], f32, tag="neg1")
            nc.vector.memset(neg1, -1.0)
            XQ = N // 4
            xbigs, xrs = [], []
            x_dmas = []
            for k in range(K):
                xb_k = []
                for q in range(4):
                    xbig = xp.tile([128, XQ], bf16, tag="xbig", bufs=2,
                                   name=f"xbig{k}_{q}")
                    x_dmas.append(nc.sync.dma_start(
                        out=xbig, in_=xT[k, :, q * XQ:(q + 1) * XQ]
                    ))
                    xb_k.append(xbig)
                xbigs.append(xb_k)
                xr_sb = xp.tile([128, ROWS], bf16, tag="xr", bufs=2,
                                name=f"xr{k}")
                nc.gpsimd.dma_start(out=xr_sb, in_=xrT[k])
                xrs.append(xr_sb)

            # ---- resident adjacency (mask in phase 1, weights in phase 2) ----
            # Stored fp8 in DRAM (binary 0/1, exact); SWDGE casts to bf16 on
            # the way into SBUF, halving the HBM stream. Chain the group loads
            # (2 in flight) so they LAND in consumption order.
            adj_sb = []  # adj_sb[g][:, j, :] is chunk g*CPG+j as [128, ROWS]
            adj_dmas = []
            for g in range(NGRP):
                at = adjp.tile([128, CPG, ROWS], bf16, tag="adjt", name=f"adj{g}")
                if ADJ_FP8:
                    d = nc.gpsimd.dma_start(out=at, in_=adjT[g])
                else:
                    d = nc.sync.dma_start(out=at, in_=adjT[g])
                if g >= 2:
                    add_dep_helper(d.ins, adj_dmas[g - 2].ins,
                                   reason="stream adjacency in order")
                else:
                    # only k=0's x quarters must beat the adjacency stream;
                    # phase-1 k=0 group 0 needs adj group 0 by ~25us
                    add_dep_helper(d.ins, x_dmas[3].ins,
                                   reason="k0 x lands before adjacency stream")
                adj_dmas.append(d)
                adj_sb.append(at)

            def adj_ch(ch):
                return adj_sb[ch // CPG][:, ch % CPG, :]

            whp = []   # [128, NCH, 65] per k: [Wh chunk | ones]
            v_sb = []  # [128, NCH] f32 per k: exp(Wh2)
            u_bc = []  # [128, ROWS] bf16 per k: exp(Wh1[rows]) bcast
            psA_cm = tc.tile_pool(name="psA", bufs=3, space="PSUM")
            psA = psA_cm.__enter__()
            for k in range(K):
                # ---- phase 0b first: u = exp(Wh1[core rows]) broadcast ----
                xr_sb = xrs[k]
                for half in range(2):
                    psu = psA.tile([1, 512], f32, tag="psu",
                                   name=f"psu{k}_{half}")
                    nc.tensor.matmul(
                        psu,
                        lhsT=waug_sb[:, 64:65],
                        rhs=xr_sb[:, half * 512:(half + 1) * 512],
                        start=True,
                        stop=True,
                    )
                    uh = sm.tile([1, 512], bf16, tag="uh", name=f"uh{k}_{half}")
                    nc.scalar.activation(uh, psu, AF.Exp)
                    nc.gpsimd.dma_start(
                        out=urow[k, :, half * 512:(half + 1) * 512], in_=uh
                    )
                ub = persist.tile([128, ROWS], bf16, tag=f"ub{k}")
                nc.gpsimd.dma_start(out=ub, in_=urow[k].to_broadcast((128, ROWS)))
                u_bc.append(ub)

                # ---- phase 0: Wh_aug = x @ [W | W@a1 | W@a2] ----
                whp_k = persist.tile([128, NCH, 65], bf16, tag=f"whp{k}")
                nc.vector.memset(whp_k[:, :, 64:65], 1.0)
                wh2_k = persist.tile([128, NCH], f32, tag=f"wh2{k}")
                for q in range(4):
                    xbig = xbigs[k][q]
                    for cb in range(0, NCH // 4, 4):
                        ps0 = psA.tile([128, 4, 66], f32, tag="ps0",
                                       name=f"ps0_{k}_{q}_{cb}")
                        for j in range(4):
                            ch = cb + j
                            nc.tensor.matmul(
                                ps0[:, j, :],
                                lhsT=xbig[:, ch * 128:(ch + 1) * 128],
                                rhs=waug_sb,
                                start=True,
                                stop=True,
                            )
                        gch = q * (NCH // 4) + cb
                        if (gch // 4) % 2 == 0:
                            nc.vector.tensor_copy(
                                whp_k[:, gch:gch + 4, 0:64], ps0[:, :, 0:64]
                            )
                        else:
                            nc.scalar.copy(
                                whp_k[:, gch:gch + 4, 0:64], ps0[:, :, 0:64]
                            )
                        nc.vector.tensor_copy(wh2_k[:, gch:gch + 4], ps0[:, :, 65])
                whp.append(whp_k)

                # v = exp(Wh2), split so phase 1 unblocks after first half
                v_k = persist.tile([128, NCH], f32, tag=f"v{k}")
                nc.scalar.activation(
                    v_k[:, 0:NCH // 2], wh2_k[:, 0:NCH // 2], AF.Exp
                )
                nc.scalar.activation(
                    v_k[:, NCH // 2:], wh2_k[:, NCH // 2:], AF.Exp
                )
                v_sb.append(v_k)
            psA_cm.__exit__(None, None, None)

            # One PSUM bank per accumulator: concurrent matmul accumulation
            # groups sharing a bank clobber each other (only the last-opened
            # group in a bank survives).
            psB_cm = tc.tile_pool(name="psB", bufs=8, space="PSUM")
            psB = psB_cm.__enter__()

            act_sched = _act_schedule()

            # ---- phase 1 for one k ----
            # p[m, r] = adj[r, m] * max(u[r] v[m], 1); h'^T chunks = p.T @ [Wh|1]
            # DVE path: t = max(u*v, 1) via tensor_scalar; ACT path:
            # t = relu(u*v - 1) on ScalarE with the missing "+ adj" term as a
            # second PE accumulation of adj @ Whp. One merged mask-multiply
            # per (k, group) applies adj to 4 chunks at once.
            ps_h = {}

            def phase1_group(k, g):
                t4 = tp.tile([128, CPG, ROWS], bf16, tag="t", name=f"t{k}_{g}")
                for j in range(CPG):
                    ch = g * CPG + j
                    if act_sched[k][ch]:
                        nc.scalar.activation(
                            t4[:, j, :], u_bc[k], AF.Relu,
                            bias=neg1, scale=v_sb[k][:, ch:ch + 1],
                        )
                    else:
                        nc.vector.tensor_scalar(
                            out=t4[:, j, :],
                            in0=u_bc[k],
                            scalar1=v_sb[k][:, ch:ch + 1],
                            scalar2=1.0,
                            op0=AO.mult,
                            op1=AO.max,
                        )
                p4 = pp.tile([128, CPG, ROWS], bf16, tag="p", name=f"p{k}_{g}")
                nc.vector.tensor_mul(p4, t4, adj_sb[g])
                for j in range(CPG):
                    ch = g * CPG + j
                    for ns in range(RCH):
                        nc.tensor.matmul(
                            ps_h[k][ns],
                            lhsT=p4[:, j, ns * 128:(ns + 1) * 128],
                            rhs=whp[k][:, ch, :],
                            start=(ch == 0),
                            stop=(ch == NCH - 1),
                        )
                    if act_sched[k][ch]:
                        for ns in range(RCH):
                            nc.tensor.matmul(
                                ps_h[k][ns],
                                lhsT=adj_ch(ch)[:, ns * 128:(ns + 1) * 128],
                                rhs=whp[k][:, ch, :],
                                start=False,
                                stop=False,
                            )

            def phase1_finish(k):
                hp_acc = hpio.tile([128, RCH, OUT_F], hp_dt, tag="hpacc", bufs=2,
                                   name=f"hpacc{k}")
                for ns in range(RCH):
                    rs = sm.tile([128, 1], f32, tag="rs", name=f"rs{k}_{ns}")
                    nc.vector.reciprocal(rs, ps_h[k][ns][:, 64:65])
                    nc.vector.tensor_scalar_mul(
                        hp_acc[:, ns, :], ps_h[k][ns][:, 0:64], rs
                    )
                hp_dma = nc.sync.dma_start if not HP_FP8 else nc.gpsimd.dma_start
                hp_dma(
                    out=hp_local[k][:].rearrange("(ns p) o -> p ns o", p=128),
                    in_=hp_acc,
                )
                nc.gpsimd.collective_compute(
                    "AllGather",
                    mybir.AluOpType.bypass,
                    replica_groups=[list(range(NCORES))],
                    ins=[hp_local[k][:]],
                    outs=[hp_full[k][:]],
                )

            # ---- phase 2 for one k, one group: out += adj[rows,:] @ h'_k ----
            # ps_o tiles allocated lazily (they reuse the psh banks after the
            # phase-1 accumulators are drained); k=0 and k=1 run sequential
            # accumulation groups in the same bank.
            ps_o = []

            def phase2_group(g):
                hpbig = hpio.tile([128, CPG, K, OUT_F], bf16, tag="hpbig",
                                  name=f"hpbig{g}")
                base = g * CPG * 128
                for k in range(K):
                    src = hp_full[k][base:base + CPG * 128, :].rearrange(
                        "(j p) o -> p j o", p=128
                    )
                    if HP_FP8:
                        nc.gpsimd.dma_start(out=hpbig[:, :, k, :], in_=src)
                    else:
                        nc.sync.dma_start(out=hpbig[:, :, k, :], in_=src)
                for j in range(CPG):
                    ch = g * CPG + j
                    for rs_ in range(RCH):
                        nc.tensor.matmul(
                            ps_o[rs_],
                            lhsT=adj_sb[g][:, j, rs_ * 128:(rs_ + 1) * 128],
                            rhs=hpbig[:, j, :, :],
                            start=(ch == 0),
                            stop=(ch == NCH - 1),
                        )

            # ---- schedule ----
            ps_h[0] = [
                psB.tile([128, 65], f32, tag="acc", name=f"psh0_{i}")
                for i in range(RCH)
            ]
            for g in range(NGRP):
                phase1_group(0, g)
            phase1_finish(0)

            ps_h[1] = [
                psB.tile([128, 65], f32, tag="acc", name=f"psh1_{i}")
                for i in range(RCH)
            ]
            for g in range(NGRP):
                phase1_group(1, g)
            phase1_finish(1)

            ps_o.extend(
                psB.tile([128, K, OUT_F], f32, tag="acc", name=f"pso{i}")
                for i in range(RCH)
            )
            for g in range(NGRP):
                phase2_group(g)

            out_acc = hpio.tile([128, RCH, K * OUT_F], f32, tag="outacc", bufs=1)
            for rs_ in range(RCH):
                nc.vector.tensor_scalar_max(
                    out_acc[:, rs_, :], ps_o[rs_], 0.0
                )
            nc.sync.dma_start(
                out=out[:].rearrange("(rs p) k o -> p rs (k o)", p=128),
                in_=out_acc,
            )
            psB_cm.__exit__(None, None, None)

    nc.finalize()
    return nc


def _prep_inputs(x, adj, weight, a):
    bf = ml_dtypes.bfloat16
    adj_np_dt = ml_dtypes.float8_e4m3 if ADJ_FP8 else bf
    w32 = weight.astype(np.float32)
    a32 = a.astype(np.float32)
    waug = np.concatenate(
        [w32, w32 @ a32[:OUT_F], w32 @ a32[OUT_F:]], axis=1
    ).astype(bf)  # [128, 66]
    xT = np.ascontiguousarray(x.astype(np.float32).transpose(0, 2, 1)).astype(bf)
    adj_t = adj.astype(adj_np_dt)
    in_maps = []
    for c in range(NCORES):
        rows = slice(c * ROWS, (c + 1) * ROWS)
        # [N, ROWS] -> [NGRP, 128, CPG, ROWS]; chunk ch = g*CPG+j sits at
        # adj^T rows ch*128 ... ch*128+128
        adjT_c = (
            np.ascontiguousarray(adj_t[rows].T)
            .reshape(NGRP, CPG, 128, ROWS)
            .transpose(0, 2, 1, 3)
        )
        adjT_c = np.ascontiguousarray(adjT_c)
        xrT_c = np.ascontiguousarray(xT[:, :, rows])
        in_maps.append({"adjT": adjT_c, "xT": xT, "xrT": xrT_c, "waug": waug})
    return in_maps


def _run(in_maps, trace=False, **kw):
    from concourse.bass_utils import run_bass_kernel_spmd

    nc = _build()
    return run_bass_kernel_spmd(nc, in_maps, list(range(NCORES)), trace=trace, **kw)


def kernel(**inputs):
    x = np.asarray(inputs["x"])
    adj = np.asarray(inputs["adj"])
    weight = np.asarray(inputs["weight"])
    a = np.asarray(inputs["a"])
    in_maps = _prep_inputs(x, adj, weight, a)
    res = _run(in_maps)
    full = np.concatenate(
        [np.asarray(res.results[c]["out"]) for c in range(NCORES)], axis=0
    )  # [N, K, OUT_F]
    return np.ascontiguousarray(full.transpose(1, 0, 2)).astype(np.float32)


# revision 31
# speedup vs baseline: 1.0776x; 1.0776x over previous
import sys
import functools

sys.path.insert(0, "/opt/trn_rl_repo")
import numpy as np
import ml_dtypes

# Problem constants (nn_Causal_GraphConvolution): hardcoded per harness contract.
K = 2
N = 8192
IN_F = 128
OUT_F = 64
NCORES = 8
ROWS = N // NCORES   # 1024 attention rows per core
NCH = N // 128       # 64 column chunks of 128
RCH = ROWS // 128    # 8 row chunks of 128
CPG = 4              # adjacency chunks per DMA group
NGRP = NCH // CPG    # 16 adjacency tiles

ADJ_FP8 = True       # adjacency stored fp8 in DRAM, cast to bf16 on DMA
HP_FP8 = False       # h' gathered in fp8 (halves the collective)
P2_SPLIT = True      # k=0 phase 2 runs while the k=1 all-gather is in flight

# t-builds sent to ScalarE (relu path, plus a PE adj@Whp correction); the
# rest stay on VectorE as tensor_scalar. Balanced so ACT(n*1075) matches
# DVE(masks 70us + (128-n)*327 + misc) inside the phase-1 window.
N_ACT = 88


def _act_schedule():
    """on_act[k][ch]: every chunk i with floor crossing gets the ACT path,
    spread evenly so neither engine starves in any stretch."""
    on_act = [[False] * NCH for _ in range(K)]
    for i in range(K * NCH):
        if (i + 1) * N_ACT // (K * NCH) > i * N_ACT // (K * NCH):
            k, ch = i % K, i // K
            if ch != NCH - 1:
                on_act[k][ch] = True
    return on_act


@functools.lru_cache(maxsize=3)
def _build():
    import concourse.bacc as bacc
    import concourse.tile as tile
    from concourse.tile import add_dep_helper
    from concourse import mybir

    bf16 = mybir.dt.bfloat16
    f32 = mybir.dt.float32
    f8e4 = mybir.dt.float8e4
    AO = mybir.AluOpType
    AF = mybir.ActivationFunctionType
    adj_dt = f8e4 if ADJ_FP8 else bf16
    hp_dt = f8e4 if HP_FP8 else bf16

    nc = bacc.Bacc(num_devices=NCORES)

    # Per-core inputs (the SPMD in_maps supply different data per core).
    # adjT[g, p, j, r] = adj[core_rows[r], (g*CPG+j)*128 + p]
    adjT = nc.declare_dram_parameter("adjT", [NGRP, 128, CPG, ROWS], adj_dt, False)
    xT = nc.declare_dram_parameter("xT", [K, IN_F, N], bf16, False)
    xrT = nc.declare_dram_parameter("xrT", [K, IN_F, ROWS], bf16, False)
    waug = nc.declare_dram_parameter("waug", [IN_F, 66], bf16, False)
    out = nc.declare_dram_parameter("out", [ROWS, K, OUT_F], f32, True)

    urow = nc.dram_tensor("urow", [K, 1, ROWS], bf16)
    hp_local = [
        nc.dram_tensor(f"hp_local{k}", [ROWS, OUT_F], hp_dt) for k in range(K)
    ]
    hp_full = [
        nc.dram_tensor(f"hp_full{k}", [N, OUT_F], hp_dt, addr_space="Shared")
        for k in range(K)
    ]

    with tile.TileContext(nc) as tc:
        with (
            tc.tile_pool(name="persist", bufs=1) as persist,
            tc.tile_pool(name="adjp", bufs=NGRP) as adjp,
            tc.tile_pool(name="xp", bufs=1) as xp,
            tc.tile_pool(name="tp", bufs=2) as tp,
            tc.tile_pool(name="pp", bufs=2) as pp,
            tc.tile_pool(name="hpio", bufs=3) as hpio,
            tc.tile_pool(name="sm", bufs=4) as sm,
        ):
            # phase-0 inputs first so they are not queued behind the 8MB
            # adjacency stream
            waug_sb = persist.tile([IN_F, 66], bf16, tag="waug")
            nc.gpsimd.dma_start(out=waug_sb, in_=waug[:])
            neg1 = persist.tile([128, 1], f32, tag="neg1")
            nc.vector.memset(neg1, -1.0)
            XQ = N // 4
            xbigs, xrs = [], []
            x_dmas = []
            for k in range(K):
                xb_k = []
                for q in range(4):
                    xbig = xp.tile([128, XQ], bf16, tag="xbig", bufs=2,
                                   name=f"xbig{k}_{q}")
                    x_dmas.append(nc.sync.dma_start(
                        out=xbig, in_=xT[k, :, q * XQ:(q + 1) * XQ]
                    ))
                    xb_k.append(xbig)
                xbigs.append(xb_k)
                xr_sb = xp.tile([128, ROWS], bf16, tag="xr", bufs=2,
                                name=f"xr{k}")
                nc.gpsimd.dma_start(out=xr_sb, in_=xrT[k])
                xrs.append(xr_sb)

            # ---- resident adjacency (mask in phase 1, weights in phase 2) ----
            # Stored fp8 in DRAM (binary 0/1, exact); SWDGE casts to bf16 on
            # the way into SBUF, halving the HBM stream. Chain the group loads
            # (2 in flight) so they LAND in consumption order.
            adj_sb = []  # adj_sb[g][:, j, :] is chunk g*CPG+j as [128, ROWS]
            adj_dmas = []
            for g in range(NGRP):
                at = adjp.tile([128, CPG, ROWS], bf16, tag="adjt", name=f"adj{g}")
                if ADJ_FP8:
                    d = nc.gpsimd.dma_start(out=at, in_=adjT[g])
                else:
                    d = nc.sync.dma_start(out=at, in_=adjT[g])
                if g >= 2:
                    add_dep_helper(d.ins, adj_dmas[g - 2].ins,
                                   reason="stream adjacency in order")
                else:
                    # only k=0's x quarters must beat the adjacency stream;
                    # phase-1 k=0 group 0 needs adj group 0 by ~25us
                    add_dep_helper(d.ins, x_dmas[3].ins,
                                   reason="k0 x lands before adjacency stream")
                adj_dmas.append(d)
                adj_sb.append(at)

            def adj_ch(ch):
                return adj_sb[ch // CPG][:, ch % CPG, :]

            whp = []   # [128, NCH, 65] per k: [Wh chunk | ones]
            v_sb = []  # [128, NCH] f32 per k: exp(Wh2)
            u_bc = []  # [128, ROWS] bf16 per k: exp(Wh1[rows]) bcast
            psA_cm = tc.tile_pool(name="psA", bufs=3, space="PSUM")
            psA = psA_cm.__enter__()
            for k in range(K):
                # ---- phase 0b first: u = exp(Wh1[core rows]) broadcast ----
                xr_sb = xrs[k]
                for half in range(2):
                    psu = psA.tile([1, 512], f32, tag="psu",
                                   name=f"psu{k}_{half}")
                    nc.tensor.matmul(
                        psu,
                        lhsT=waug_sb[:, 64:65],
                        rhs=xr_sb[:, half * 512:(half + 1) * 512],
                        start=True,
                        stop=True,
                    )
                    uh = sm.tile([1, 512], bf16, tag="uh", name=f"uh{k}_{half}")
                    nc.scalar.activation(uh, psu, AF.Exp)
                    nc.gpsimd.dma_start(
                        out=urow[k, :, half * 512:(half + 1) * 512], in_=uh
                    )
                ub = persist.tile([128, ROWS], bf16, tag=f"ub{k}")
                nc.gpsimd.dma_start(out=ub, in_=urow[k].to_broadcast((128, ROWS)))
                u_bc.append(ub)

                # ---- phase 0: Wh_aug = x @ [W | W@a1 | W@a2] ----
                whp_k = persist.tile([128, NCH, 65], bf16, tag=f"whp{k}")
                nc.vector.memset(whp_k[:, :, 64:65], 1.0)
                wh2_k = persist.tile([128, NCH], f32, tag=f"wh2{k}")
                for q in range(4):
                    xbig = xbigs[k][q]
                    for cb in range(0, NCH // 4, 4):
                        ps0 = psA.tile([128, 4, 66], f32, tag="ps0",
                                       name=f"ps0_{k}_{q}_{cb}")
                        for j in range(4):
                            ch = cb + j
                            nc.tensor.matmul(
                                ps0[:, j, :],
                                lhsT=xbig[:, ch * 128:(ch + 1) * 128],
                                rhs=waug_sb,
                                start=True,
                                stop=True,
                            )
                        gch = q * (NCH // 4) + cb
                        if (gch // 4) % 2 == 0:
                            nc.vector.tensor_copy(
                                whp_k[:, gch:gch + 4, 0:64], ps0[:, :, 0:64]
                            )
                        else:
                            nc.scalar.copy(
                                whp_k[:, gch:gch + 4, 0:64], ps0[:, :, 0:64]
                            )
                        nc.vector.tensor_copy(wh2_k[:, gch:gch + 4], ps0[:, :, 65])
                whp.append(whp_k)

                # v = exp(Wh2), split so phase 1 unblocks after first half
                v_k = persist.tile([128, NCH], f32, tag=f"v{k}")
                nc.scalar.activation(
                    v_k[:, 0:NCH // 2], wh2_k[:, 0:NCH // 2], AF.Exp
                )
                nc.scalar.activation(
                    v_k[:, NCH // 2:], wh2_k[:, NCH // 2:], AF.Exp
                )
                v_sb.append(v_k)
            psA_cm.__exit__(None, None, None)

            # One PSUM bank per accumulator: concurrent matmul accumulation
            # groups sharing a bank clobber each other (only the last-opened
            # group in a bank survives).
            psB_cm = tc.tile_pool(name="psB", bufs=8, space="PSUM")
            psB = psB_cm.__enter__()

            act_sched = _act_schedule()

            # ---- phase 1 for one k ----
            # p[m, r] = adj[r, m] * max(u[r] v[m], 1); h'^T chunks = p.T @ [Wh|1]
            # DVE path: t = max(u*v, 1) via tensor_scalar; ACT path:
            # t = relu(u*v - 1) on ScalarE with the missing "+ adj" term as a
            # second PE accumulation of adj @ Whp. One merged mask-multiply
            # per (k, group) applies adj to 4 chunks at once.
            ps_h = {}

            def phase1_group(k, g):
                t4 = tp.tile([128, CPG, ROWS], bf16, tag="t", name=f"t{k}_{g}")
                for j in range(CPG):
                    ch = g * CPG + j
                    if act_sched[k][ch]:
                        nc.scalar.activation(
                            t4[:, j, :], u_bc[k], AF.Relu,
                            bias=neg1, scale=v_sb[k][:, ch:ch + 1],
                        )
                    else:
                        nc.vector.tensor_scalar(
                            out=t4[:, j, :],
                            in0=u_bc[k],
                            scalar1=v_sb[k][:, ch:ch + 1],
                            scalar2=1.0,
                            op0=AO.mult,
                            op1=AO.max,
                        )
                p4 = pp.tile([128, CPG, ROWS], bf16, tag="p", name=f"p{k}_{g}")
                nc.vector.tensor_mul(p4, t4, adj_sb[g])
                for j in range(CPG):
                    ch = g * CPG + j
                    for ns in range(RCH):
                        nc.tensor.matmul(
                            ps_h[k][ns],
                            lhsT=p4[:, j, ns * 128:(ns + 1) * 128],
                            rhs=whp[k][:, ch, :],
                            start=(ch == 0),
                            stop=(ch == NCH - 1),
                        )
                    if act_sched[k][ch]:
                        for ns in range(RCH):
                            nc.tensor.matmul(
                                ps_h[k][ns],
                                lhsT=adj_ch(ch)[:, ns * 128:(ns + 1) * 128],
                                rhs=whp[k][:, ch, :],
                                start=False,
                                stop=False,
                            )

            def phase1_finish(k):
                hp_acc = hpio.tile([128, RCH, OUT_F], hp_dt, tag="hpacc", bufs=2,
                                   name=f"hpacc{k}")
                for ns in range(RCH):
                    rs = sm.tile([128, 1], f32, tag="rs", name=f"rs{k}_{ns}")
                    nc.vector.reciprocal(rs, ps_h[k][ns][:, 64:65])
                    nc.vector.tensor_scalar_mul(
                        hp_acc[:, ns, :], ps_h[k][ns][:, 0:64], rs
                    )
                hp_dma = nc.sync.dma_start if not HP_FP8 else nc.gpsimd.dma_start
                hp_dma(
                    out=hp_local[k][:].rearrange("(ns p) o -> p ns o", p=128),
                    in_=hp_acc,
                )
                nc.gpsimd.collective_compute(
                    "AllGather",
                    mybir.AluOpType.bypass,
                    replica_groups=[list(range(NCORES))],
                    ins=[hp_local[k][:]],
                    outs=[hp_full[k][:]],
                )

            # ---- phase 2 for one k, one group: out += adj[rows,:] @ h'_k ----
            # ps_o tiles allocated lazily (they reuse the psh banks after the
            # phase-1 accumulators are drained); k=0 and k=1 run sequential
            # accumulation groups in the same bank.
            ps_o = []

            def phase2_group(k, g):
                # per-k so the k=0 half's matmuls fill the otherwise-dead
                # bubble while the k=1 all-gather is in flight
                hpbig = hpio.tile([128, CPG, OUT_F], bf16, tag="hpbig",
                                  name=f"hpbig{k}_{g}")
                base = g * CPG * 128
                src = hp_full[k][base:base + CPG * 128, :].rearrange(
                    "(j p) o -> p j o", p=128
                )
                nc.sync.dma_start(out=hpbig, in_=src)
                for j in range(CPG):
                    ch = g * CPG + j
                    for rs_ in range(RCH):
                        nc.tensor.matmul(
                            ps_o[rs_][:, k, :],
                            lhsT=adj_sb[g][:, j, rs_ * 128:(rs_ + 1) * 128],
                            rhs=hpbig[:, j, :],
                            start=(ch == 0),
                            stop=(ch == NCH - 1),
                        )

            # ---- schedule ----
            ps_h[0] = [
                psB.tile([128, 65], f32, tag="acc", name=f"psh0_{i}")
                for i in range(RCH)
            ]
            for g in range(NGRP):
                phase1_group(0, g)
            phase1_finish(0)

            ps_h[1] = [
                psB.tile([128, 65], f32, tag="acc", name=f"psh1_{i}")
                for i in range(RCH)
            ]
            for g in range(NGRP):
                phase1_group(1, g)
            phase1_finish(1)

            ps_o.extend(
                psB.tile([128, K, OUT_F], f32, tag="acc", name=f"pso{i}")
                for i in range(RCH)
            )
            for g in range(NGRP):
                phase2_group(0, g)
            for g in range(NGRP):
                phase2_group(1, g)

            out_acc = hpio.tile([128, RCH, K * OUT_F], f32, tag="outacc", bufs=1)
            for rs_ in range(RCH):
                nc.vector.tensor_scalar_max(
                    out_acc[:, rs_, :], ps_o[rs_], 0.0
                )
            nc.sync.dma_start(
                out=out[:].rearrange("(rs p) k o -> p rs (k o)", p=128),
                in_=out_acc,
            )
            psB_cm.__exit__(None, None, None)

    nc.finalize()
    return nc


def _prep_inputs(x, adj, weight, a):
    bf = ml_dtypes.bfloat16
    adj_np_dt = ml_dtypes.float8_e4m3 if ADJ_FP8 else bf
    w32 = weight.astype(np.float32)
    a32 = a.astype(np.float32)
    waug = np.concatenate(
        [w32, w32 @ a32[:OUT_F], w32 @ a32[OUT_F:]], axis=1
    ).astype(bf)  # [128, 66]
    xT = np.ascontiguousarray(x.astype(np.float32).transpose(0, 2, 1)).astype(bf)
    adj_t = adj.astype(adj_np_dt)
    in_maps = []
    for c in range(NCORES):
        rows = slice(c * ROWS, (c + 1) * ROWS)
        # [N, ROWS] -> [NGRP, 128, CPG, ROWS]; chunk ch = g*CPG+j sits at
        # adj^T rows ch*128 ... ch*128+128
        adjT_c = (
            np.ascontiguousarray(adj_t[rows].T)
            .reshape(NGRP, CPG, 128, ROWS)
            .transpose(0, 2, 1, 3)
        )
        adjT_c = np.ascontiguousarray(adjT_c)
        xrT_c = np.ascontiguousarray(xT[:, :, rows])
        in_maps.append({"adjT": adjT_c, "xT": xT, "xrT": xrT_c, "waug": waug})
    return in_maps


def _run(in_maps, trace=False, **kw):
    from concourse.bass_utils import run_bass_kernel_spmd

    nc = _build()
    return run_bass_kernel_spmd(nc, in_maps, list(range(NCORES)), trace=trace, **kw)


def kernel(**inputs):
    x = np.asarray(inputs["x"])
    adj = np.asarray(inputs["adj"])
    weight = np.asarray(inputs["weight"])
    a = np.asarray(inputs["a"])
    in_maps = _prep_inputs(x, adj, weight, a)
    res = _run(in_maps)
    full = np.concatenate(
        [np.asarray(res.results[c]["out"]) for c in range(NCORES)], axis=0
    )  # [N, K, OUT_F]
    return np.ascontiguousarray(full.transpose(1, 0, 2)).astype(np.float32)
